# revision 12
# baseline (speedup 1.0000x reference)
"""Trainium2 Bass kernel for ConvspatialAttentionBlock.

Computes, per batch b:
  q = Wq @ x + bq            [64, N]
  k = Wk @ x + bk            [64, N]
  v = Wv @ x + bv            [512, N]
  P = softmax(q^T k, axis=j) [N, N]
  out = gamma * (v @ P^T) + x

Sharding: 8 cores = (batch b in 0..3) x (query-half h in 0..1). Each core
computes attention output for its 2048 query positions against all 4096
keys of its batch.

Host<->device traffic is minimized (the axon tunnel runs at ~40 MB/s,
half-duplex, which dominates wall-clock):
  - each core uploads ONLY its query half of x, quantized to int8 with
    per-channel scales (host-computed over the full row so both pair
    members dequantize identically); the two cores of a batch exchange
    halves on-device with an HBM AllGather over pairs. Key/value columns
    are order-agnostic in softmax+AV, so both pair members can use the
    gathered [h0|h1] layout; queries always come from the core's own
    input tensor -> no rank-dependent addressing. Dequantized to fp16 on
    the ACT engine (scale is a per-partition operand).
  - the projection weights are packed host-side into wpack [C, 640] =
    [WqT | WkT | gamma*WvT] fp16; each core uploads a 64-row slice and an
    AllGather over all 8 cores reconstructs the full matrix on-device.
  - the device returns gamma*read (NOT the +x residual; the host adds the
    exact f32 x instead), quantized to int8 with exact per-channel
    per-query-chunk scales computed on the DVE, shipped as a tiny f32
    side tensor.

Device algebra (per core), attention matmuls in float32r (full PE rate):
  gamma and bv are folded host-side: Wv' = gamma*Wv, bv' = gamma*bv, so
  r[c,i] = (sum_j v'_raw[c,j] e[j,i]) / den[i] + bv'[c]   (out = r + x)
  where e = exp(logits^T - 4) (shift keeps e under fp16 max; softmax is
  shift-invariant), den[i] = sum_j e[j,i] accumulated on the PE via
  ones-vector matmuls.
"""

import numpy as np

import concourse.bacc as bacc
import concourse.mybir as mybir
import concourse.tile as tile

B, C, N = 4, 512, 4096
D = 64            # query/key channels (C//8)
NQ = N // 2       # queries per core
NCORES = 8
IC = 512          # query-chunk (free dim per matmul)
NIC = NQ // IC    # 4 query chunks
NJT = N // 128    # 32 key tiles
CCH = C // 128    # 4 channel chunks
WCOL = 2 * D + C  # 640 packed weight columns
WSL = C // NCORES  # 64 weight rows per core

F16 = mybir.dt.float16
I8 = mybir.dt.int8
F32 = mybir.dt.float32
F32R = mybir.dt.float32r
ACT_COPY = mybir.ActivationFunctionType.Copy
ACT_EXP = mybir.ActivationFunctionType.Exp
ACT_IDENT = mybir.ActivationFunctionType.Identity


def build():
    nc = bacc.Bacc("TRN2", target_bir_lowering=False, debug=False,
                   num_devices=NCORES)

    xh_d = nc.dram_tensor("xh", [C, NQ], I8, kind="ExternalInput")
    wsl_d = nc.dram_tensor("wsl", [WSL, WCOL], F16, kind="ExternalInput")
    # aux = [bq(64) | bk(64) | bvs(512) | xsc(512)] packed into one tensor
    aux_d = nc.dram_tensor("aux", [2 * D + 2 * C, 1], F32,
                           kind="ExternalInput")
    # out columns [0:NQ) = int8 gamma*read; [NQ:NQ+4*NIC) = bitcast f32
    # per-channel per-chunk quantization maxes
    out_d = nc.dram_tensor("out", [C, NQ + 4 * NIC], I8,
                           kind="ExternalOutput")

    with tile.TileContext(nc) as tc:
        with (
            tc.tile_pool(name="dram", bufs=1, space="DRAM") as dp,
            tc.tile_pool(name="persist", bufs=1) as pp,
            tc.tile_pool(name="work", bufs=3) as wp,
            tc.tile_pool(name="fin", bufs=2) as fp,
            tc.tile_pool(name="ps2", bufs=4, space="PSUM") as ps2,
            tc.tile_pool(name="ps1", bufs=1, space="PSUM") as ps1,
        ):
            # ---- collective prologue: reconstruct weights + peer x ----
            wsl_b = dp.tile([WSL, WCOL], F16, tag="wsl_b", name="wsl_b")
            wg = dp.tile([C, WCOL], F16, tag="wg", name="wg")
            # x exchange in two column-halves (contiguous bounce tiles)
            # so K/V work on the first half overlaps the second transfer
            NH = NQ // 2
            xh_bs = [dp.tile([C, NH], I8, tag=f"xh_b{g}", name=f"xh_b{g}")
                     for g in range(2)]
            xgs = [dp.tile([2 * C, NH], I8, tag=f"xg{g}", name=f"xg{g}")
                   for g in range(2)]
            nc.gpsimd.dma_start(wsl_b[:], wsl_d.ap())
            for g in range(2):
                nc.gpsimd.dma_start(
                    xh_bs[g][:], xh_d.ap()[:, g * NH:(g + 1) * NH])
            pairs = [[2 * p, 2 * p + 1] for p in range(NCORES // 2)]
            nc.gpsimd.collective_compute(
                "AllGather", mybir.AluOpType.bypass,
                replica_groups=[list(range(NCORES))],
                ins=[wsl_b.opt()], outs=[wg.opt()])
            for g in range(2):
                nc.gpsimd.collective_compute(
                    "AllGather", mybir.AluOpType.bypass, replica_groups=pairs,
                    ins=[xh_bs[g].opt()], outs=[xgs[g].opt()])

            # ---- persistent SBUF ----
            # xq: this core's query half, direct from the input (no
            # collective dependency). xk: all N key columns from the
            # gathered pair buffer ([h0|h1] on both pair members).
            # int8 staging tiles, dequantized to fp16 on the ACT engine.
            xq8_t = pp.tile([128, CCH, NQ], I8, tag="xq8")
            for cc in range(CCH):
                nc.sync.dma_start(
                    xq8_t[:, cc, :], xh_d.ap()[cc * 128:(cc + 1) * 128, :])
            xsc_t = pp.tile([128, CCH], F32, tag="xsc")
            nc.sync.dma_start(
                xsc_t[:], aux_d.ap()[2 * D + C:2 * D + 2 * C, :]
                .rearrange("(a p) b -> p (a b)", p=128))

            bq_t = pp.tile([D, 1], F32, tag="bq")
            nc.sync.dma_start(bq_t[:], aux_d.ap()[0:D, :])
            bk_t = pp.tile([D, 1], F32, tag="bk")
            nc.sync.dma_start(bk_t[:], aux_d.ap()[D:2 * D, :])
            bvs_t = pp.tile([128, CCH], F32, tag="bvs")
            nc.sync.dma_start(
                bvs_t[:], aux_d.ap()[2 * D:2 * D + C, :]
                .rearrange("(a p) b -> p (a b)", p=128))
            onesc_t = pp.tile([128, 1], F32, tag="onesc")
            nc.gpsimd.memset(onesc_t[:], 1.0)
            neg2_t = pp.tile([128, 1], F32, tag="neg2")
            nc.gpsimd.memset(neg2_t[:], -4.0)

            wq_t = pp.tile([128, CCH, D], F16, tag="wq")
            nc.sync.dma_start(
                wq_t[:], wg[:, 0:D].rearrange("(a p) d -> p a d", p=128))
            wk_t = pp.tile([128, CCH, D], F16, tag="wk")
            nc.sync.dma_start(
                wk_t[:], wg[:, D:2 * D].rearrange("(a p) d -> p a d", p=128))
            wv_t = pp.tile([128, CCH, C], F16, tag="wv")
            for cc in range(CCH):
                nc.sync.dma_start(
                    wv_t[:, cc, :],
                    wg[cc * 128:(cc + 1) * 128, 2 * D:WCOL])

            # key/value source: 4 column-groups (gather-half g x rank r);
            # key order is irrelevant to softmax+AV
            xk8_t = pp.tile([128, CCH, N], I8, tag="xk8")
            for g in range(2):
                for r in range(2):
                    for cc in range(CCH):
                        nc.sync.dma_start(
                            xk8_t[:, cc, (2 * g + r) * NH:
                                  (2 * g + r + 1) * NH],
                            xgs[g][r * C + cc * 128:
                                   r * C + (cc + 1) * 128, :])

            # dequantize: x = x8 * xsc[c]  (per-partition scale operand)
            xq_t = pp.tile([128, CCH, NQ], F16, tag="xq")
            for cc in range(CCH):
                nc.scalar.activation(
                    xq_t[:, cc, :], xq8_t[:, cc, :], ACT_COPY,
                    scale=xsc_t[:, cc:cc + 1])
            xk_t = pp.tile([128, CCH, N], F16, tag="xk")
            for gr in range(4):
                for cc in range(CCH):
                    nc.scalar.activation(
                        xk_t[:, cc, gr * NH:(gr + 1) * NH],
                        xk8_t[:, cc, gr * NH:(gr + 1) * NH], ACT_COPY,
                        scale=xsc_t[:, cc:cc + 1])

            q_t = pp.tile([D, NQ], F32R, tag="q")
            k_t = pp.tile([D, N], F32R, tag="k")
            # f16 so the AV matmul runs on same-width operands (the backend
            # rejects f32r x f16); ex is also f16, with exp shifted by -4 so
            # e^(max logit) stays under f16's 65504 (softmax shift-invariant)
            vt_t = pp.tile([128, NJT, C], F16, tag="vt")

            # ---- phase A: projections ----
            # q[d, i] from this core's own half (ready first)
            for icq in range(NIC):
                ps = ps2.tile([128, IC], F32, tag="lg", name="pa_ps")
                for cc in range(CCH):
                    nc.tensor.matmul(
                        ps[:D, :], wq_t[:, cc, :],
                        xq_t[:, cc, icq * IC:(icq + 1) * IC],
                        start=(cc == 0), stop=(cc == CCH - 1))
                nc.scalar.activation(
                    q_t[:, icq * IC:(icq + 1) * IC], ps[:D, :],
                    ACT_IDENT, bias=bq_t[:])
            # k[d, j] over all gathered columns
            for jc in range(N // IC):
                ps = ps2.tile([128, IC], F32, tag="lg", name="pa_ps")
                for cc in range(CCH):
                    nc.tensor.matmul(
                        ps[:D, :], wk_t[:, cc, :],
                        xk_t[:, cc, jc * IC:(jc + 1) * IC],
                        start=(cc == 0), stop=(cc == CCH - 1))
                nc.scalar.activation(
                    k_t[:, jc * IC:(jc + 1) * IC], ps[:D, :],
                    ACT_IDENT, bias=bk_t[:])
            # vT[j, c] = sum_ch x[ch, j] * WvT'[ch, c]
            for jt in range(NJT):
                ps = ps2.tile([128, C], F32, tag="lg", name="pv_ps")
                for cc in range(CCH):
                    nc.tensor.matmul(
                        ps[:], xk_t[:, cc, jt * 128:(jt + 1) * 128],
                        wv_t[:, cc, :],
                        start=(cc == 0), stop=(cc == CCH - 1))
                nc.scalar.activation(vt_t[:, jt, :], ps[:], ACT_COPY)

            # ---- phase B: attention, one query-chunk at a time ----
            # The PE part of each chunk's epilogue (denominator reduce) and
            # the normalize/output stage are deferred into the next chunk's
            # j-loop so the PE never sits in the reciprocal chain.
            def emit_epilogue(ep):
                ic, asb, dar = ep
                den = ps2.tile([1, IC], F32, tag="lg", name="den")
                nc.tensor.matmul(den[:], onesc_t[:].bitcast(F32R), dar[:],
                                 start=True, stop=True)
                den_sb = wp.tile([1, IC], F32, tag="den_sb", name="den_sb", bufs=1)
                nc.scalar.activation(den_sb[:], den[:], ACT_COPY)
                rec = wp.tile([1, IC], F32, tag="rec", name="rec", bufs=1)
                nc.vector.reciprocal(rec[:], den_sb[:])
                rdbc = fp.tile([128, IC], F32, tag="rdbc", name="rdbc", bufs=1)
                nc.gpsimd.partition_broadcast(rdbc[:], rec[:])
                # r[c, i] = av[c, i] * rdbc[i] + bvs[c]; int8-quantize with
                # an exact per-channel (per-chunk) scale: o8 = r * 127/max|r|
                for ct in range(CCH):
                    nc.vector.tensor_mul(asb[ct][:], asb[ct][:], rdbc[:])
                    nc.vector.tensor_scalar_add(
                        asb[ct][:], asb[ct][:], bvs_t[:, ct:ct + 1])
                    cm = wp.tile([128, 1], F32, tag="cm", name="cm", bufs=4)
                    nc.vector.tensor_reduce(
                        cm[:], asb[ct][:], mybir.AxisListType.X,
                        mybir.AluOpType.max, apply_absolute_value=True)
                    nc.vector.tensor_scalar_max(cm[:], cm[:], 1e-30)
                    rs = wp.tile([128, 1], F32, tag="rs", name="rs", bufs=4)
                    nc.vector.reciprocal(rs[:], cm[:])
                    nc.vector.tensor_scalar_mul(rs[:], rs[:], 127.0)
                    o = fp.tile([128, IC], I8, tag="o", name="o", bufs=4)
                    nc.vector.tensor_scalar_mul(o[:], asb[ct][:], rs[:])
                    nc.sync.dma_start(
                        out_d.ap()[ct * 128:(ct + 1) * 128,
                                   ic * IC:(ic + 1) * IC],
                        o[:])
                    nc.sync.dma_start(
                        out_d.ap()[ct * 128:(ct + 1) * 128,
                                   NQ + 4 * ic:NQ + 4 * (ic + 1)],
                        cm[:].bitcast(I8))

            pending = None
            for ic in range(NIC):
                av = [ps1.tile([128, IC], F32, tag=f"av{ct}", name=f"av{ct}")
                      for ct in range(CCH)]
                dacc = wp.tile([128, IC], F32, tag="dacc", name="dacc", bufs=1)
                qs = q_t[:, ic * IC:(ic + 1) * IC]
                for jt in range(NJT):
                    lg = ps2.tile([128, IC], F32, tag="lg", name="lg")
                    nc.tensor.matmul(
                        lg[:], k_t[:, jt * 128:(jt + 1) * 128], qs,
                        start=True, stop=True)
                    ex = wp.tile([128, IC], F16, tag="ex", name="ex", bufs=5)
                    nc.scalar.activation(ex[:], lg[:], ACT_EXP, bias=neg2_t[:])
                    # denominator partial sums on DVE (partition-wise)
                    if jt == 0:
                        nc.vector.tensor_copy(dacc[:], ex[:])
                    else:
                        nc.vector.tensor_add(dacc[:], dacc[:], ex[:])
                    for ct in range(CCH):
                        nc.tensor.matmul(
                            av[ct][:], vt_t[:, jt, ct * 128:(ct + 1) * 128],
                            ex[:],
                            start=(jt == 0), stop=(jt == NJT - 1))
                    if jt == 3 and pending is not None:
                        emit_epilogue(pending)
                        pending = None
                # drain av banks to SBUF promptly (split over DVE and ACT)
                # so the next chunk's matmuls can reuse the banks at once
                asb = []
                for ct in range(CCH):
                    a = fp.tile([128, IC], F32, tag=f"asb{ct}",
                                name=f"asb{ct}", bufs=1)
                    if ct % 2 == 0:
                        nc.vector.tensor_copy(a[:], av[ct][:])
                    else:
                        nc.scalar.activation(a[:], av[ct][:], ACT_COPY)
                    asb.append(a)
                dar = wp.tile([128, IC], F32R, tag="dar", name="dar", bufs=1)
                nc.scalar.activation(dar[:], dacc[:], ACT_COPY)
                pending = (ic, asb, dar)
            emit_epilogue(pending)
    nc.compile()
    return nc


_RUNNER = None


def _get_runner():
    """Build the Bass program once and return a reusable jitted SPMD runner."""
    global _RUNNER
    if _RUNNER is not None:
        return _RUNNER

    import jax
    from jax.sharding import Mesh, PartitionSpec
    from jax.experimental.shard_map import shard_map
    from concourse import bass2jax
    from concourse import mybir as _mybir

    nc = build()
    bass2jax.install_neuronx_cc_hook()

    partition_name = (nc.partition_id_tensor.name
                      if nc.partition_id_tensor else None)
    in_names = []
    out_names = []
    out_avals = []
    for alloc in nc.m.functions[0].allocations:
        if not isinstance(alloc, _mybir.MemoryLocationSet):
            continue
        if alloc.kind == "ExternalInput":
            name = alloc.memorylocations[0].name
            if name != partition_name:
                in_names.append(name)
        elif alloc.kind == "ExternalOutput":
            out_names.append(alloc.memorylocations[0].name)
            out_avals.append(jax.core.ShapedArray(
                tuple(alloc.tensor_shape), _mybir.dt.np(alloc.dtype)))
    n_params = len(in_names)
    all_names = list(in_names)
    if partition_name is not None:
        all_names.append(partition_name)

    def _body(*args):
        operands = list(args)
        if partition_name is not None:
            operands.append(bass2jax.partition_id_tensor())
        outs = bass2jax._bass_exec_p.bind(
            *operands,
            out_avals=tuple(out_avals),
            in_names=tuple(all_names),
            out_names=tuple(out_names),
            lowering_input_output_aliases=(),
            sim_require_finite=True,
            sim_require_nnan=True,
            nc=nc,
        )
        return tuple(outs)

    devices = jax.devices()[:NCORES]
    mesh = Mesh(np.asarray(devices), ("core",))
    in_specs = (PartitionSpec("core"),) * n_params
    out_specs = (PartitionSpec("core"),) * len(out_names)
    sharded = jax.jit(
        shard_map(_body, mesh=mesh, in_specs=in_specs, out_specs=out_specs,
                  check_rep=False),
        keep_unused=True)

    from concurrent.futures import ThreadPoolExecutor
    pool = ThreadPoolExecutor(NCORES)

    def run(in_maps):
        concat_in = [
            np.concatenate([np.asarray(m[name]) for m in in_maps], axis=0)
            for name in in_names
        ]
        out_arrs = sharded(*concat_in)
        # fetch shards in parallel (higher aggregate D2H rate than one
        # serial np.asarray over the tunnel)
        fetched = []
        for i, a in enumerate(out_arrs):
            shards = sorted(a.addressable_shards, key=lambda s: s.index)
            parts = list(pool.map(lambda s: np.asarray(s.data), shards))
            fetched.append(parts)
        return [
            {name: fetched[i][c]
             for i, name in enumerate(out_names)}
            for c in range(NCORES)
        ]

    _RUNNER = (run, nc)
    return _RUNNER


def make_in_maps(minibatch, Wq, bq, Wk, bk, Wv, bv, gamma):
    gamma0 = float(np.asarray(gamma).reshape(-1)[0])
    wpack = np.concatenate(
        [np.asarray(Wq, np.float32).T,
         np.asarray(Wk, np.float32).T,
         (gamma0 * np.asarray(Wv, np.float32)).T],
        axis=1).astype(np.float16)  # [C, 640]
    bq2 = np.asarray(bq, np.float32).reshape(D, 1)
    bk2 = np.asarray(bk, np.float32).reshape(D, 1)
    bvs = (gamma0 * np.asarray(bv, np.float32)).reshape(C, 1)
    mb = np.asarray(minibatch, np.float32)
    # int8 per-channel quantization of x; scales over the FULL row so both
    # pair members dequantize the shared columns identically
    xsc_full = np.maximum(np.abs(mb).max(axis=2), 1e-6) / 127.0  # [B, C]
    x8_full = np.rint(mb / xsc_full[:, :, None]).astype(np.int8)
    in_maps = []
    for core in range(NCORES):
        b, h = divmod(core, 2)
        xh = np.ascontiguousarray(x8_full[b][:, h * NQ:(h + 1) * NQ])
        aux = np.concatenate(
            [bq2, bk2, bvs,
             xsc_full[b].reshape(C, 1).astype(np.float32)], axis=0)
        wsl = np.ascontiguousarray(wpack[core * WSL:(core + 1) * WSL])
        in_maps.append(dict(xh=xh, wsl=wsl, aux=aux))
    return in_maps


def kernel(minibatch, Wq, bq, Wk, bk, Wv, bv, gamma):
    run, _ = _get_runner()
    in_maps = make_in_maps(minibatch, Wq, bq, Wk, bk, Wv, bv, gamma)
    results = run(in_maps)
    out = np.empty((B, C, N), np.float32)
    mb = np.asarray(minibatch, np.float32)
    for core in range(NCORES):
        b, h = divmod(core, 2)
        packed = results[core]["out"]                     # [C, NQ+4*NIC]
        o8 = packed[:, :NQ].astype(np.float32)
        osc = np.ascontiguousarray(
            packed[:, NQ:]).view(np.float32) / 127.0      # [C, NIC]
        r = o8 * np.repeat(osc, IC, axis=1)               # gamma*read
        out[b][:, h * NQ:(h + 1) * NQ] = r + mb[b][:, h * NQ:(h + 1) * NQ]
    return out


# revision 13
# speedup vs baseline: 1.0535x; 1.0535x over previous
"""Trainium2 Bass kernel for ConvspatialAttentionBlock.

Computes, per batch b:
  q = Wq @ x + bq            [64, N]
  k = Wk @ x + bk            [64, N]
  v = Wv @ x + bv            [512, N]
  P = softmax(q^T k, axis=j) [N, N]
  out = gamma * (v @ P^T) + x

Sharding: 8 cores = (batch b in 0..3) x (query-half h in 0..1). Each core
computes attention output for its 2048 query positions against all 4096
keys of its batch.

Host<->device traffic is minimized (the axon tunnel runs at ~40 MB/s,
half-duplex, which dominates wall-clock):
  - each core uploads ONLY its query half of x, quantized to int8 with
    per-channel scales (host-computed over the full row so both pair
    members dequantize identically); the two cores of a batch exchange
    halves on-device with an HBM AllGather over pairs. Key/value columns
    are order-agnostic in softmax+AV, so both pair members can use the
    gathered [h0|h1] layout; queries always come from the core's own
    input tensor -> no rank-dependent addressing. Dequantized to fp16 on
    the ACT engine (scale is a per-partition operand).
  - the projection weights are packed host-side into wpack [C, 640] =
    [WqT | WkT | gamma*WvT] fp16; each core uploads a 64-row slice and an
    AllGather over all 8 cores reconstructs the full matrix on-device.
  - the device returns gamma*read (NOT the +x residual; the host adds the
    exact f32 x instead), quantized to int8 with exact per-channel
    per-query-chunk scales computed on the DVE, shipped as a tiny f32
    side tensor.

Device algebra (per core), attention matmuls in float32r (full PE rate):
  gamma and bv are folded host-side: Wv' = gamma*Wv, bv' = gamma*bv, so
  r[c,i] = (sum_j v'_raw[c,j] e[j,i]) / den[i] + bv'[c]   (out = r + x)
  where e = exp(logits^T - 4) (shift keeps e under fp16 max; softmax is
  shift-invariant), den[i] = sum_j e[j,i] accumulated on the PE via
  ones-vector matmuls.
"""

import numpy as np

import concourse.bacc as bacc
import concourse.mybir as mybir
import concourse.tile as tile

B, C, N = 4, 512, 4096
D = 64            # query/key channels (C//8)
NQ = N // 2       # queries per core
NCORES = 8
IC = 512          # query-chunk (free dim per matmul)
NIC = NQ // IC    # 4 query chunks
NJT = N // 128    # 32 key tiles
CCH = C // 128    # 4 channel chunks
WCOL = 2 * D + C  # 640 packed weight columns
WSL = C // NCORES  # 64 weight rows per core

F16 = mybir.dt.float16
I8 = mybir.dt.int8
F32 = mybir.dt.float32
F32R = mybir.dt.float32r
ACT_COPY = mybir.ActivationFunctionType.Copy
ACT_EXP = mybir.ActivationFunctionType.Exp
ACT_IDENT = mybir.ActivationFunctionType.Identity


def build():
    nc = bacc.Bacc("TRN2", target_bir_lowering=False, debug=False,
                   num_devices=NCORES)

    xh_d = nc.dram_tensor("xh", [C, NQ], I8, kind="ExternalInput")
    wsl_d = nc.dram_tensor("wsl", [WSL, WCOL], F16, kind="ExternalInput")
    # aux = [bq(64) | bk(64) | bvs(512) | xsc(512)] packed into one tensor
    aux_d = nc.dram_tensor("aux", [2 * D + 2 * C, 1], F32,
                           kind="ExternalInput")
    # out columns [0:NQ) = int8 gamma*read; [NQ:NQ+4*NIC) = bitcast f32
    # per-channel per-chunk quantization maxes
    out_d = nc.dram_tensor("out", [C, NQ + 4 * NIC], I8,
                           kind="ExternalOutput")

    with tile.TileContext(nc) as tc:
        with (
            tc.tile_pool(name="dram", bufs=1, space="DRAM") as dp,
            tc.tile_pool(name="persist", bufs=1) as pp,
            tc.tile_pool(name="work", bufs=3) as wp,
            tc.tile_pool(name="fin", bufs=2) as fp,
            tc.tile_pool(name="ps2", bufs=4, space="PSUM") as ps2,
            tc.tile_pool(name="ps1", bufs=1, space="PSUM") as ps1,
        ):
            # ---- collective prologue: reconstruct weights + peer x ----
            wsl_b = dp.tile([WSL, WCOL], F16, tag="wsl_b", name="wsl_b")
            wg = dp.tile([C, WCOL], F16, tag="wg", name="wg")
            # x exchange in two column-halves (contiguous bounce tiles)
            # so K/V work on the first half overlaps the second transfer
            NH = NQ // 2
            xh_bs = [dp.tile([C, NH], I8, tag=f"xh_b{g}", name=f"xh_b{g}")
                     for g in range(2)]
            xgs = [dp.tile([2 * C, NH], I8, tag=f"xg{g}", name=f"xg{g}")
                   for g in range(2)]
            nc.gpsimd.dma_start(wsl_b[:], wsl_d.ap())
            for g in range(2):
                nc.gpsimd.dma_start(
                    xh_bs[g][:], xh_d.ap()[:, g * NH:(g + 1) * NH])
            pairs = [[2 * p, 2 * p + 1] for p in range(NCORES // 2)]
            nc.gpsimd.collective_compute(
                "AllGather", mybir.AluOpType.bypass,
                replica_groups=[list(range(NCORES))],
                ins=[wsl_b.opt()], outs=[wg.opt()])
            for g in range(2):
                nc.gpsimd.collective_compute(
                    "AllGather", mybir.AluOpType.bypass, replica_groups=pairs,
                    ins=[xh_bs[g].opt()], outs=[xgs[g].opt()])

            # ---- persistent SBUF ----
            # xq: this core's query half, direct from the input (no
            # collective dependency). xk: all N key columns from the
            # gathered pair buffer ([h0|h1] on both pair members).
            # int8 staging tiles, dequantized to fp16 on the ACT engine.
            xq8_t = pp.tile([128, CCH, NQ], I8, tag="xq8")
            for cc in range(CCH):
                nc.sync.dma_start(
                    xq8_t[:, cc, :], xh_d.ap()[cc * 128:(cc + 1) * 128, :])
            xsc_t = pp.tile([128, CCH], F32, tag="xsc")
            nc.sync.dma_start(
                xsc_t[:], aux_d.ap()[2 * D + C:2 * D + 2 * C, :]
                .rearrange("(a p) b -> p (a b)", p=128))

            bq_t = pp.tile([D, 1], F32, tag="bq")
            nc.sync.dma_start(bq_t[:], aux_d.ap()[0:D, :])
            bk_t = pp.tile([D, 1], F32, tag="bk")
            nc.sync.dma_start(bk_t[:], aux_d.ap()[D:2 * D, :])
            bvs_t = pp.tile([128, CCH], F32, tag="bvs")
            nc.sync.dma_start(
                bvs_t[:], aux_d.ap()[2 * D:2 * D + C, :]
                .rearrange("(a p) b -> p (a b)", p=128))
            onesc_t = pp.tile([128, 1], F32, tag="onesc")
            nc.gpsimd.memset(onesc_t[:], 1.0)
            eshift_t = pp.tile([128, 1], F32, tag="eshift")
            nc.gpsimd.memset(eshift_t[:], -4.0)

            wq_t = pp.tile([128, CCH, D], F16, tag="wq")
            nc.sync.dma_start(
                wq_t[:], wg[:, 0:D].rearrange("(a p) d -> p a d", p=128))
            wk_t = pp.tile([128, CCH, D], F16, tag="wk")
            nc.sync.dma_start(
                wk_t[:], wg[:, D:2 * D].rearrange("(a p) d -> p a d", p=128))
            wv_t = pp.tile([128, CCH, C], F16, tag="wv")
            for cc in range(CCH):
                nc.sync.dma_start(
                    wv_t[:, cc, :],
                    wg[cc * 128:(cc + 1) * 128, 2 * D:WCOL])

            # key/value source: 4 column-groups (gather-half g x rank r);
            # key order is irrelevant to softmax+AV
            xk8_t = pp.tile([128, CCH, N], I8, tag="xk8")
            for g in range(2):
                for r in range(2):
                    for cc in range(CCH):
                        nc.sync.dma_start(
                            xk8_t[:, cc, (2 * g + r) * NH:
                                  (2 * g + r + 1) * NH],
                            xgs[g][r * C + cc * 128:
                                   r * C + (cc + 1) * 128, :])

            # dequantize: x = x8 * xsc[c]  (per-partition scale operand)
            xq_t = pp.tile([128, CCH, NQ], F16, tag="xq")
            for cc in range(CCH):
                nc.scalar.activation(
                    xq_t[:, cc, :], xq8_t[:, cc, :], ACT_COPY,
                    scale=xsc_t[:, cc:cc + 1])
            xk_t = pp.tile([128, CCH, N], F16, tag="xk")
            for gr in range(4):
                for cc in range(CCH):
                    nc.scalar.activation(
                        xk_t[:, cc, gr * NH:(gr + 1) * NH],
                        xk8_t[:, cc, gr * NH:(gr + 1) * NH], ACT_COPY,
                        scale=xsc_t[:, cc:cc + 1])

            q_t = pp.tile([D, NQ], F32R, tag="q")
            k_t = pp.tile([D, N], F32R, tag="k")
            # f16 so the AV matmul runs on same-width operands (the backend
            # rejects f32r x f16); ex is also f16, with exp shifted by -4 so
            # e^(max logit) stays under f16's 65504 (softmax shift-invariant)
            vt_t = pp.tile([128, NJT, C], F16, tag="vt")

            # ---- phase A: projections ----
            # q[d, i] from this core's own half (ready first)
            for icq in range(NIC):
                ps = ps2.tile([128, IC], F32, tag="lg", name="pa_ps")
                for cc in range(CCH):
                    nc.tensor.matmul(
                        ps[:D, :], wq_t[:, cc, :],
                        xq_t[:, cc, icq * IC:(icq + 1) * IC],
                        start=(cc == 0), stop=(cc == CCH - 1))
                nc.scalar.activation(
                    q_t[:, icq * IC:(icq + 1) * IC], ps[:D, :],
                    ACT_IDENT, bias=bq_t[:])
            # k[d, j] over all gathered columns
            for jc in range(N // IC):
                ps = ps2.tile([128, IC], F32, tag="lg", name="pa_ps")
                for cc in range(CCH):
                    nc.tensor.matmul(
                        ps[:D, :], wk_t[:, cc, :],
                        xk_t[:, cc, jc * IC:(jc + 1) * IC],
                        start=(cc == 0), stop=(cc == CCH - 1))
                nc.scalar.activation(
                    k_t[:, jc * IC:(jc + 1) * IC], ps[:D, :],
                    ACT_IDENT, bias=bk_t[:])
            # vT[j, c] = sum_ch x[ch, j] * WvT'[ch, c]
            for jt in range(NJT):
                ps = ps2.tile([128, C], F32, tag="lg", name="pv_ps")
                for cc in range(CCH):
                    nc.tensor.matmul(
                        ps[:], xk_t[:, cc, jt * 128:(jt + 1) * 128],
                        wv_t[:, cc, :],
                        start=(cc == 0), stop=(cc == CCH - 1))
                nc.scalar.activation(vt_t[:, jt, :], ps[:], ACT_COPY)

            # ---- phase B: attention, one query-chunk at a time ----
            # The PE part of each chunk's epilogue (denominator reduce) and
            # the normalize/output stage are deferred into the next chunk's
            # j-loop so the PE never sits in the reciprocal chain.
            def emit_epilogue(ep):
                ic, asb, dar = ep
                den = ps2.tile([1, IC], F32, tag="lg", name="den")
                nc.tensor.matmul(den[:], onesc_t[:].bitcast(F32R), dar[:],
                                 start=True, stop=True)
                den_sb = wp.tile([1, IC], F32, tag="den_sb", name="den_sb", bufs=1)
                nc.scalar.activation(den_sb[:], den[:], ACT_COPY)
                rec = wp.tile([1, IC], F32, tag="rec", name="rec", bufs=1)
                nc.vector.reciprocal(rec[:], den_sb[:])
                rdbc = fp.tile([128, IC], F32, tag="rdbc", name="rdbc", bufs=1)
                nc.gpsimd.partition_broadcast(rdbc[:], rec[:])
                # r[c, i] = av[c, i] * rdbc[i] + bvs[c]; int8-quantize with
                # an exact per-channel (per-chunk) scale: o8 = r * 127/max|r|
                for ct in range(CCH):
                    nc.vector.tensor_mul(asb[ct][:], asb[ct][:], rdbc[:])
                    nc.vector.tensor_scalar_add(
                        asb[ct][:], asb[ct][:], bvs_t[:, ct:ct + 1])
                    cm = wp.tile([128, 1], F32, tag="cm", name="cm", bufs=4)
                    nc.vector.tensor_reduce(
                        cm[:], asb[ct][:], mybir.AxisListType.X,
                        mybir.AluOpType.max, apply_absolute_value=True)
                    nc.vector.tensor_scalar_max(cm[:], cm[:], 1e-30)
                    rs = wp.tile([128, 1], F32, tag="rs", name="rs", bufs=4)
                    nc.vector.reciprocal(rs[:], cm[:])
                    nc.vector.tensor_scalar_mul(rs[:], rs[:], 127.0)
                    o = fp.tile([128, IC], I8, tag="o", name="o", bufs=4)
                    nc.vector.tensor_scalar_mul(o[:], asb[ct][:], rs[:])
                    nc.sync.dma_start(
                        out_d.ap()[ct * 128:(ct + 1) * 128,
                                   ic * IC:(ic + 1) * IC],
                        o[:])
                    nc.sync.dma_start(
                        out_d.ap()[ct * 128:(ct + 1) * 128,
                                   NQ + 4 * ic:NQ + 4 * (ic + 1)],
                        cm[:].bitcast(I8))

            pending = None
            for ic in range(NIC):
                av = [ps1.tile([128, IC], F32, tag=f"av{ct}", name=f"av{ct}")
                      for ct in range(CCH)]
                dacc = wp.tile([128, IC], F32, tag="dacc", name="dacc", bufs=1)
                qs = q_t[:, ic * IC:(ic + 1) * IC]
                for jt in range(NJT):
                    lg = ps2.tile([128, IC], F32, tag="lg", name="lg")
                    nc.tensor.matmul(
                        lg[:], k_t[:, jt * 128:(jt + 1) * 128], qs,
                        start=True, stop=True)
                    ex = wp.tile([128, IC], F16, tag="ex", name="ex", bufs=5)
                    nc.scalar.activation(ex[:], lg[:], ACT_EXP, bias=eshift_t[:])
                    # denominator partial sums on DVE (partition-wise)
                    if jt == 0:
                        nc.vector.tensor_copy(dacc[:], ex[:])
                    else:
                        nc.vector.tensor_add(dacc[:], dacc[:], ex[:])
                    for ct in range(CCH):
                        nc.tensor.matmul(
                            av[ct][:], vt_t[:, jt, ct * 128:(ct + 1) * 128],
                            ex[:],
                            start=(jt == 0), stop=(jt == NJT - 1))
                    if jt == 3 and pending is not None:
                        emit_epilogue(pending)
                        pending = None
                # drain av banks to SBUF promptly (split over DVE and ACT)
                # so the next chunk's matmuls can reuse the banks at once
                asb = []
                for ct in range(CCH):
                    a = fp.tile([128, IC], F32, tag=f"asb{ct}",
                                name=f"asb{ct}", bufs=1)
                    if ct % 2 == 0:
                        nc.vector.tensor_copy(a[:], av[ct][:])
                    else:
                        nc.scalar.activation(a[:], av[ct][:], ACT_COPY)
                    asb.append(a)
                dar = wp.tile([128, IC], F32R, tag="dar", name="dar", bufs=1)
                nc.scalar.activation(dar[:], dacc[:], ACT_COPY)
                pending = (ic, asb, dar)
            emit_epilogue(pending)
    nc.compile()
    return nc


_RUNNER = None


def _get_runner():
    """Build the Bass program once and return a reusable jitted SPMD runner."""
    global _RUNNER
    if _RUNNER is not None:
        return _RUNNER

    import jax
    from jax.sharding import Mesh, PartitionSpec
    from jax.experimental.shard_map import shard_map
    from concourse import bass2jax
    from concourse import mybir as _mybir

    nc = build()
    bass2jax.install_neuronx_cc_hook()

    partition_name = (nc.partition_id_tensor.name
                      if nc.partition_id_tensor else None)
    in_names = []
    out_names = []
    out_avals = []
    for alloc in nc.m.functions[0].allocations:
        if not isinstance(alloc, _mybir.MemoryLocationSet):
            continue
        if alloc.kind == "ExternalInput":
            name = alloc.memorylocations[0].name
            if name != partition_name:
                in_names.append(name)
        elif alloc.kind == "ExternalOutput":
            out_names.append(alloc.memorylocations[0].name)
            out_avals.append(jax.core.ShapedArray(
                tuple(alloc.tensor_shape), _mybir.dt.np(alloc.dtype)))
    n_params = len(in_names)
    all_names = list(in_names)
    if partition_name is not None:
        all_names.append(partition_name)

    def _body(*args):
        operands = list(args)
        if partition_name is not None:
            operands.append(bass2jax.partition_id_tensor())
        outs = bass2jax._bass_exec_p.bind(
            *operands,
            out_avals=tuple(out_avals),
            in_names=tuple(all_names),
            out_names=tuple(out_names),
            lowering_input_output_aliases=(),
            sim_require_finite=True,
            sim_require_nnan=True,
            nc=nc,
        )
        return tuple(outs)

    devices = jax.devices()[:NCORES]
    mesh = Mesh(np.asarray(devices), ("core",))
    in_specs = (PartitionSpec("core"),) * n_params
    out_specs = (PartitionSpec("core"),) * len(out_names)
    sharded = jax.jit(
        shard_map(_body, mesh=mesh, in_specs=in_specs, out_specs=out_specs,
                  check_rep=False),
        keep_unused=True)

    from concurrent.futures import ThreadPoolExecutor
    pool = ThreadPoolExecutor(NCORES)

    def run(in_maps):
        concat_in = [
            np.concatenate([np.asarray(m[name]) for m in in_maps], axis=0)
            for name in in_names
        ]
        out_arrs = sharded(*concat_in)
        # fetch shards in parallel (higher aggregate D2H rate than one
        # serial np.asarray over the tunnel)
        fetched = []
        for i, a in enumerate(out_arrs):
            shards = sorted(a.addressable_shards, key=lambda s: s.index)
            parts = list(pool.map(lambda s: np.asarray(s.data), shards))
            fetched.append(parts)
        return [
            {name: fetched[i][c]
             for i, name in enumerate(out_names)}
            for c in range(NCORES)
        ]

    _RUNNER = (run, nc)
    return _RUNNER


def make_in_maps(minibatch, Wq, bq, Wk, bk, Wv, bv, gamma):
    gamma0 = float(np.asarray(gamma).reshape(-1)[0])
    wpack = np.concatenate(
        [np.asarray(Wq, np.float32).T,
         np.asarray(Wk, np.float32).T,
         (gamma0 * np.asarray(Wv, np.float32)).T],
        axis=1).astype(np.float16)  # [C, 640]
    bq2 = np.asarray(bq, np.float32).reshape(D, 1)
    bk2 = np.asarray(bk, np.float32).reshape(D, 1)
    bvs = (gamma0 * np.asarray(bv, np.float32)).reshape(C, 1)
    mb = np.asarray(minibatch, np.float32)
    # int8 per-channel quantization of x; scales over the FULL row so both
    # pair members dequantize the shared columns identically
    xsc_full = np.maximum(np.abs(mb).max(axis=2), 1e-6) / 127.0  # [B, C]
    x8_full = np.rint(mb / xsc_full[:, :, None]).astype(np.int8)
    in_maps = []
    for core in range(NCORES):
        b, h = divmod(core, 2)
        xh = np.ascontiguousarray(x8_full[b][:, h * NQ:(h + 1) * NQ])
        aux = np.concatenate(
            [bq2, bk2, bvs,
             xsc_full[b].reshape(C, 1).astype(np.float32)], axis=0)
        wsl = np.ascontiguousarray(wpack[core * WSL:(core + 1) * WSL])
        in_maps.append(dict(xh=xh, wsl=wsl, aux=aux))
    return in_maps


def kernel(minibatch, Wq, bq, Wk, bk, Wv, bv, gamma):
    run, _ = _get_runner()
    in_maps = make_in_maps(minibatch, Wq, bq, Wk, bk, Wv, bv, gamma)
    results = run(in_maps)
    out = np.empty((B, C, N), np.float32)
    mb = np.asarray(minibatch, np.float32)
    for core in range(NCORES):
        b, h = divmod(core, 2)
        packed = results[core]["out"]                     # [C, NQ+4*NIC]
        o8 = packed[:, :NQ].astype(np.float32)
        osc = np.ascontiguousarray(
            packed[:, NQ:]).view(np.float32) / 127.0      # [C, NIC]
        r = o8 * np.repeat(osc, IC, axis=1)               # gamma*read
        out[b][:, h * NQ:(h + 1) * NQ] = r + mb[b][:, h * NQ:(h + 1) * NQ]
    return out
